# revision 8
# baseline (speedup 1.0000x reference)
"""Trainium2 Bass kernel for nn_BidirectionalDecoder.

Strategy (8 NeuronCores):
  Kernel 1 (batch-parallel): each core gets B/8 batches. Computes Bahdanau
  attention (score matmul in float32r at full PE rate, two-pass softmax with
  SBUF-resident encT), context via DVE mult+reduce, and the zero-state
  bidirectional LSTM step. Host supplies enc transposed per batch
  ([ENC, S] layout) with values pre-rounded to the f32r grid.
  Kernel 2 (vocab-parallel): each core computes logits[:, shard] for all
  batches from the gathered LSTM outputs; fc bias added on host.

All matmuls run in float32r (fp32 storage, ~11 mantissa bits, 1 cycle/row).
"""
import os
import numpy as np

import concourse.bacc as bacc
import concourse.mybir as mybir
from concourse import tile
from concourse.bass_utils import run_bass_kernel_spmd

f32 = mybir.dt.float32
f32r = mybir.dt.float32r

NCORES = 8
SBLK = 512

LAST_RESULTS = {}

_CACHE = {}


def _round_f32r(a):
    """Round float32 array to the f32r grid (11 mantissa bits) in place-ish."""
    b = np.ascontiguousarray(a, dtype=np.float32).copy()
    v = b.view(np.uint32)
    v += 0x800
    v &= 0xFFFFF000
    return b


def _build_kernel1(bp, S, ENC, U, INDIM):
    """bp: batches per core. Returns compiled Bacc."""
    EC = ENC // 128   # e-chunks
    UC = U // 128     # u-chunks
    KC = INDIM // 128  # in_dim chunks for LSTM
    NSB = S // SBLK
    G = 3 * U         # i,g,o per direction
    Tanh = mybir.ActivationFunctionType.Tanh
    Sigmoid = mybir.ActivationFunctionType.Sigmoid
    Exp = mybir.ActivationFunctionType.Exp
    Identity = mybir.ActivationFunctionType.Identity

    nc = bacc.Bacc(None)
    ENCT = nc.dram_tensor("ENCT", [bp, ENC, S], f32r, kind="ExternalInput")
    W1 = nc.dram_tensor("W1", [ENC, U], f32r, kind="ExternalInput")
    W23 = nc.dram_tensor("W23", [ENC, U], f32r, kind="ExternalInput")
    CCT = nc.dram_tensor("CCT", [ENC, bp], f32r, kind="ExternalInput")
    BQ = nc.dram_tensor("BQ", [128, UC], f32, kind="ExternalInput")
    V4 = nc.dram_tensor("V4", [128, UC], f32r, kind="ExternalInput")
    XET = nc.dram_tensor("XET", [128, (INDIM - ENC) // 128 * bp], f32,
                         kind="ExternalInput")
    KL = nc.dram_tensor("KL", [INDIM, 2 * G], f32r, kind="ExternalInput")
    BL = nc.dram_tensor("BL", [1, 2 * G], f32, kind="ExternalInput")
    HF = nc.dram_tensor("HF", [bp, U], f32, kind="ExternalOutput")
    CF = nc.dram_tensor("CF", [bp, U], f32, kind="ExternalOutput")
    HB = nc.dram_tensor("HB", [bp, U], f32, kind="ExternalOutput")
    CB = nc.dram_tensor("CB", [bp, U], f32, kind="ExternalOutput")

    with tile.TileContext(nc) as tc:
        with tc.tile_pool(name="wpool", bufs=1) as wp:
            w1_r = wp.tile([128, EC, U], f32r, tag="w1")
            nc.sync.dma_start(out=w1_r[:], in_=W1.rearrange("(c p) u -> p c u", p=128))
            v_r = wp.tile([128, UC], f32r, tag="v4")
            nc.sync.dma_start(out=v_r[:], in_=V4[:])
            bq_sb = wp.tile([128, UC], f32, tag="bq")
            nc.sync.dma_start(out=bq_sb[:], in_=BQ[:])
            qt_sb = wp.tile([128, UC * bp], f32, tag="qt")
            xin_f = wp.tile([128, KC * bp], f32, tag="xinf")
            nc.sync.dma_start(out=xin_f[:, EC * bp:KC * bp], in_=XET[:])
            bl_sb = wp.tile([1, 2 * G], f32, tag="bl")
            nc.sync.dma_start(out=bl_sb[:], in_=BL[:])
            bias_bc = wp.tile([bp, 2 * G], f32, tag="blbc")
            nc.gpsimd.partition_broadcast(bias_bc[:], bl_sb[:])

            # ---- phase 1: qT = (W23.T @ ccT) + bq  -> [U, bp] as UC chunks
            with tc.tile_pool(name="p1", bufs=1) as p1, \
                 tc.tile_pool(name="psQ", bufs=2, space="PSUM") as psQ:
                w23_r = p1.tile([128, EC, U], f32r, tag="w23")
                nc.sync.dma_start(out=w23_r[:],
                                  in_=W23.rearrange("(c p) u -> p c u", p=128))
                cct_r = p1.tile([128, EC, bp], f32r, tag="cct")
                nc.sync.dma_start(out=cct_r[:],
                                  in_=CCT.rearrange("(c p) b -> p c b", p=128))
                for uc in range(UC):
                    pq = psQ.tile([128, bp], f32, tag="psq")
                    for kc in range(EC):
                        nc.tensor.matmul(pq[:], w23_r[:, kc, uc * 128:(uc + 1) * 128],
                                         cct_r[:, kc, :],
                                         start=(kc == 0), stop=(kc == EC - 1))
                    nc.scalar.activation(qt_sb[:, uc * bp:(uc + 1) * bp], pq[:],
                                         Identity, bias=bq_sb[:, uc:uc + 1],
                                         scale=1.0)

            # ---- phase 2: attention per batch
            with tc.tile_pool(name="encp", bufs=NSB + 2) as encp, \
                 tc.tile_pool(name="thp", bufs=6) as thp, \
                 tc.tile_pool(name="prodp", bufs=1) as prodp, \
                 tc.tile_pool(name="pbcp", bufs=2) as pbcp, \
                 tc.tile_pool(name="rowp", bufs=3) as rowp, \
                 tc.tile_pool(name="smp", bufs=6) as smp, \
                 tc.tile_pool(name="psA", bufs=2, space="PSUM") as psA, \
                 tc.tile_pool(name="psS", bufs=2, space="PSUM") as psS:
                for b in range(bp):
                    enc_tiles = []
                    for sb_i in range(NSB):
                        et = encp.tile([128, EC, SBLK], f32r, tag="enc")
                        src = ENCT[b].rearrange("(c p) s -> p c s", p=128)
                        nc.sync.dma_start(
                            out=et[:], in_=src[:, :, sb_i * SBLK:(sb_i + 1) * SBLK])
                        enc_tiles.append(et)
                    scores = rowp.tile([1, S], f32, tag="row")
                    for sb_i in range(NSB):
                        ths = []
                        for uc in range(UC):
                            ph = psA.tile([128, SBLK], f32, tag="psh")
                            for kc in range(EC):
                                nc.tensor.matmul(
                                    ph[:], w1_r[:, kc, uc * 128:(uc + 1) * 128],
                                    enc_tiles[sb_i][:, kc, :],
                                    start=(kc == 0), stop=(kc == EC - 1))
                            th = thp.tile([128, SBLK], f32r, tag="th")
                            nc.scalar.activation(
                                th[:], ph[:], Tanh,
                                bias=qt_sb[:, uc * bp + b:uc * bp + b + 1], scale=1.0)
                            ths.append(th)
                        pss = psS.tile([1, SBLK], f32, tag="pss")
                        for uc in range(UC):
                            nc.tensor.matmul(pss[:], v_r[:, uc:uc + 1], ths[uc][:],
                                             start=(uc == 0), stop=(uc == UC - 1))
                        nc.scalar.copy(scores[:, sb_i * SBLK:(sb_i + 1) * SBLK],
                                       pss[:])
                    # softmax over S (partition 0 row)
                    mx = smp.tile([1, 1], f32, tag="mx")
                    nc.vector.tensor_reduce(out=mx[:], in_=scores[:],
                                            axis=mybir.AxisListType.X,
                                            op=mybir.AluOpType.max, negate=True)
                    prow = rowp.tile([1, S], f32, tag="row")
                    es = smp.tile([1, 1], f32, tag="es")
                    nc.scalar.activation(prow[:], scores[:], Exp, bias=mx[:],
                                         scale=1.0, accum_out=es[:])
                    rinv = smp.tile([1, 1], f32, tag="rinv")
                    nc.vector.reciprocal(rinv[:], es[:])
                    pn = rowp.tile([1, S], f32, tag="row")
                    nc.scalar.mul(pn[:], prow[:], rinv[:])
                    p_bc = pbcp.tile([128, S], f32, tag="pbc")
                    nc.gpsimd.partition_broadcast(p_bc[:], pn[:])
                    # ctx: for each e-chunk, rowsum(enc * p) -> xin column
                    for ec in range(EC):
                        prod = prodp.tile([128, S], f32, tag="prod")
                        for sb_i in range(NSB):
                            nc.vector.tensor_tensor(
                                out=prod[:, sb_i * SBLK:(sb_i + 1) * SBLK],
                                in0=enc_tiles[sb_i][:, ec, :],
                                in1=p_bc[:, sb_i * SBLK:(sb_i + 1) * SBLK],
                                op=mybir.AluOpType.mult)
                        nc.vector.tensor_reduce(
                            out=xin_f[:, ec * bp + b:ec * bp + b + 1], in_=prod[:],
                            axis=mybir.AxisListType.X, op=mybir.AluOpType.add)

            # ---- phase 3: LSTM (both directions; f-gate skipped, c0=0)
            xin_r = wp.tile([128, KC * bp], f32r, tag="xinr")
            nc.scalar.copy(xin_r[:], xin_f[:])
            with tc.tile_pool(name="klp", bufs=3) as klp, \
                 tc.tile_pool(name="gp", bufs=16) as gp, \
                 tc.tile_pool(name="psZ", bufs=2, space="PSUM") as psZ:
                kl_src = KL.rearrange("(c p) n -> p c n", p=128)
                gates = []
                for nb in range(2 * G // SBLK):
                    pz = psZ.tile([bp, SBLK], f32, tag="psz")
                    for kc in range(KC):
                        kt = klp.tile([128, SBLK], f32r, tag="kl")
                        nc.sync.dma_start(
                            out=kt[:],
                            in_=kl_src[:, kc, nb * SBLK:(nb + 1) * SBLK])
                        nc.tensor.matmul(pz[:], xin_r[:, kc * bp:(kc + 1) * bp],
                                         kt[:], start=(kc == 0), stop=(kc == KC - 1))
                    z_sb = gp.tile([bp, SBLK], f32, tag="g")
                    nc.vector.tensor_tensor(
                        out=z_sb[:], in0=pz[:],
                        in1=bias_bc[:, nb * SBLK:(nb + 1) * SBLK],
                        op=mybir.AluOpType.add)
                    gates.append(z_sb)
                # gate layout per direction: i (U), g (U), o (U); U = SBLK*UCg
                UCg = U // SBLK

                def gact(idx, func):
                    out = []
                    for j in range(UCg):
                        t = gp.tile([bp, SBLK], f32, tag="g")
                        nc.scalar.activation(t[:], gates[idx * UCg + j][:], func,
                                             scale=1.0)
                        out.append(t)
                    return out

                for d, (HH, CC) in enumerate(((HF, CF), (HB, CB))):
                    off = d * 3
                    i_t = gact(off + 0, Sigmoid)
                    g_t = gact(off + 1, Tanh)
                    o_t = gact(off + 2, Sigmoid)
                    for j in range(UCg):
                        c_t = gp.tile([bp, SBLK], f32, tag="g")
                        nc.vector.tensor_tensor(out=c_t[:], in0=i_t[j][:],
                                                in1=g_t[j][:],
                                                op=mybir.AluOpType.mult)
                        nc.sync.dma_start(
                            out=CC[:, j * SBLK:(j + 1) * SBLK], in_=c_t[:])
                        tc_t = gp.tile([bp, SBLK], f32, tag="g")
                        nc.scalar.activation(tc_t[:], c_t[:], Tanh, scale=1.0)
                        h_t = gp.tile([bp, SBLK], f32, tag="g")
                        nc.vector.tensor_tensor(out=h_t[:], in0=o_t[j][:],
                                                in1=tc_t[:],
                                                op=mybir.AluOpType.mult)
                        nc.sync.dma_start(
                            out=HH[:, j * SBLK:(j + 1) * SBLK], in_=h_t[:])

    nc.compile()
    return nc


def _build_kernel2(B, D2, vsh):
    """Logits kernel: [B, vsh] = outT.T @ fc_shard. D2 = 2*U."""
    KC = D2 // 128
    nc = bacc.Bacc(None)
    OUTT = nc.dram_tensor("OUTT", [D2, B], f32r, kind="ExternalInput")
    FCW = nc.dram_tensor("FCW", [D2, vsh], f32r, kind="ExternalInput")
    LG = nc.dram_tensor("LG", [B, vsh], f32, kind="ExternalOutput")
    nnb = (vsh + SBLK - 1) // SBLK
    with tile.TileContext(nc) as tc:
        with tc.tile_pool(name="op", bufs=1) as op, \
             tc.tile_pool(name="fp", bufs=3) as fp, \
             tc.tile_pool(name="ob", bufs=3) as ob, \
             tc.tile_pool(name="psL", bufs=4, space="PSUM") as psL:
            outt_r = op.tile([128, KC, B], f32r, tag="outt")
            nc.sync.dma_start(out=outt_r[:],
                              in_=OUTT.rearrange("(c p) b -> p c b", p=128))
            fc_src = FCW.rearrange("(c p) v -> p c v", p=128)
            for nb in range(nnb):
                n0 = nb * SBLK
                w = min(vsh, n0 + SBLK) - n0
                ft = fp.tile([128, KC, SBLK], f32r, tag="fc")
                nc.sync.dma_start(out=ft[:, :, :w], in_=fc_src[:, :, n0:n0 + w])
                pl = psL.tile([B, SBLK], f32, tag="psl")
                for kc in range(KC):
                    nc.tensor.matmul(pl[:, :w], outt_r[:, kc, :], ft[:, kc, :w],
                                     start=(kc == 0), stop=(kc == KC - 1))
                o_sb = ob.tile([B, SBLK], f32, tag="o")
                nc.vector.tensor_copy(o_sb[:, :w], pl[:, :w])
                nc.sync.dma_start(out=LG[:, n0:n0 + w], in_=o_sb[:, :w])
    nc.compile()
    return nc


def prepare1(x, c_fwd, c_bwd, enc_output, emb,
             W1_w, W1_b, W2_w, W2_b, W3_w, W3_b, V_w, V_b,
             Kf, bf, Kb, bb, fc_w, fc_b):
    """Host prep + kernel-1 build. Returns (nc1, in_maps, dims)."""
    x = np.asarray(x)
    c_fwd = np.asarray(c_fwd, dtype=np.float32)
    c_bwd = np.asarray(c_bwd, dtype=np.float32)
    enc_output = np.asarray(enc_output, dtype=np.float32)
    emb = np.asarray(emb, dtype=np.float32)
    W1_w = np.asarray(W1_w, dtype=np.float32)
    fc_w = np.asarray(fc_w, dtype=np.float32)

    B, S, ENC = enc_output.shape
    U = W1_w.shape[1]
    EDIM = emb.shape[1]
    INDIM = ENC + EDIM
    VSZ = fc_w.shape[1]
    bp = B // NCORES
    vsh = VSZ // NCORES
    UC = U // 128
    EC = ENC // 128

    # ---------- host prep ----------
    # encT per batch, rounded to f32r grid
    encT = _round_f32r(np.ascontiguousarray(enc_output.transpose(0, 2, 1)))
    # embedding gather
    xe = emb[x[:, 0].astype(np.int64)]                      # [B, EDIM]
    xin_tail = xe.T                                         # [EDIM, B]
    # hidden projection inputs: [c_fwd | c_bwd] @ [[W2],[W3]]
    cct = np.concatenate([c_fwd, c_bwd], axis=1).T          # [2U, B] = [ENC, B]
    w23 = np.concatenate([np.asarray(W2_w, np.float32),
                          np.asarray(W3_w, np.float32)], axis=0)  # [2U, U]
    bq = (np.asarray(W1_b, np.float32) + np.asarray(W2_b, np.float32)
          + np.asarray(W3_b, np.float32))                   # [U]
    bq_2d = np.ascontiguousarray(bq.reshape(UC, 128).T)     # [128, UC]
    v4 = np.ascontiguousarray(np.asarray(V_w, np.float32).reshape(UC, 128).T)
    # LSTM weights: keep i, g, o columns (f-gate multiplies c0=0)
    def igo(K):
        K = np.asarray(K, np.float32)
        return np.concatenate([K[:, 0:U], K[:, 2 * U:3 * U], K[:, 3 * U:4 * U]],
                              axis=1)
    kl = np.concatenate([igo(Kf), igo(Kb)], axis=1)         # [INDIM, 6U]
    def igo_b(bv):
        bv = np.asarray(bv, np.float32)
        return np.concatenate([bv[0:U], bv[2 * U:3 * U], bv[3 * U:4 * U]])
    blrow = np.concatenate([igo_b(bf), igo_b(bb)])[None, :]  # [1, 6U]

    w1_r = _round_f32r(W1_w)
    w23_r = _round_f32r(w23)
    cct_r = _round_f32r(cct)
    v4_r = _round_f32r(v4)
    kl_r = _round_f32r(kl)

    key1 = ("k1", bp, S, ENC, U, INDIM)
    if key1 not in _CACHE:
        _CACHE[key1] = _build_kernel1(bp, S, ENC, U, INDIM)
    nc1 = _CACHE[key1]

    ecdim = (INDIM - ENC) // 128  # xe chunks
    in_maps = []
    for c in range(NCORES):
        sl = slice(c * bp, (c + 1) * bp)
        xet = np.ascontiguousarray(
            xin_tail[:, sl].reshape(ecdim, 128, bp).transpose(1, 0, 2)
        ).reshape(128, ecdim * bp)
        in_maps.append({
            "ENCT": encT[sl],
            "W1": w1_r,
            "W23": w23_r,
            "CCT": np.ascontiguousarray(cct_r[:, sl]),
            "BQ": bq_2d,
            "V4": v4_r,
            "XET": xet,
            "KL": kl_r,
            "BL": blrow,
        })
    return nc1, in_maps, (B, U, vsh)


def prepare2(fc_w, hf, hb, B, U, vsh):
    """Kernel-2 build + in_maps from gathered LSTM outputs."""
    fc_w = np.asarray(fc_w, dtype=np.float32)
    out = np.concatenate([hf, hb], axis=1)                  # [B, 2U]
    outT_r = _round_f32r(np.ascontiguousarray(out.T))       # [2U, B]
    key2 = ("k2", B, 2 * U, vsh)
    if key2 not in _CACHE:
        _CACHE[key2] = _build_kernel2(B, 2 * U, vsh)
    nc2 = _CACHE[key2]
    in_maps2 = []
    for c in range(NCORES):
        fcs = _round_f32r(np.ascontiguousarray(fc_w[:, c * vsh:(c + 1) * vsh]))
        in_maps2.append({"OUTT": outT_r, "FCW": fcs})
    return nc2, in_maps2


def kernel(**inputs):
    nc1, in_maps, (B, U, vsh) = prepare1(**inputs)
    res1 = run_bass_kernel_spmd(nc1, in_maps, list(range(NCORES)))
    LAST_RESULTS["k1"] = res1

    hf = np.concatenate([res1.results[c]["HF"] for c in range(NCORES)], axis=0)
    cf = np.concatenate([res1.results[c]["CF"] for c in range(NCORES)], axis=0)
    hb = np.concatenate([res1.results[c]["HB"] for c in range(NCORES)], axis=0)
    cb = np.concatenate([res1.results[c]["CB"] for c in range(NCORES)], axis=0)

    nc2, in_maps2 = prepare2(inputs["fc_w"], hf, hb, B, U, vsh)
    res2 = run_bass_kernel_spmd(nc2, in_maps2, list(range(NCORES)))
    LAST_RESULTS["k2"] = res2

    logits = np.concatenate([res2.results[c]["LG"] for c in range(NCORES)],
                            axis=1)
    logits = logits + np.asarray(inputs["fc_b"], np.float32)[None, :]
    return (logits.astype(np.float32), hf, cf, hb, cb)


# revision 15
# speedup vs baseline: 1.1803x; 1.1803x over previous
"""Trainium2 Bass kernel for nn_BidirectionalDecoder.

Strategy (8 NeuronCores):
  Kernel 1 (batch-parallel): each core gets B/8 batches. Computes Bahdanau
  attention (score matmul at 1 cycle/row), two-pass softmax with
  SBUF-resident encT, context via DVE mult+reduce, and the zero-state
  bidirectional LSTM step. Host supplies enc transposed per batch
  ([ENC, S] layout).
  Kernel 2 (vocab-parallel): each core computes logits[:, shard] for all
  batches from the gathered LSTM outputs; fc bias added on host.

KERNEL_DT=bf16 (default): enc/weights in bf16 — fastest, ~3e-3 rel err.
KERNEL_DT=f32r: fp32 storage rounded to 11 mantissa bits — ~2e-4 rel err.
"""
import os
import numpy as np
import ml_dtypes

import concourse.bacc as bacc
import concourse.mybir as mybir
from concourse import tile
from concourse.bass_utils import run_bass_kernel_spmd

f32 = mybir.dt.float32
f32r = mybir.dt.float32r
bf16 = mybir.dt.bfloat16

NCORES = 8
SBLK = 512

LAST_RESULTS = {}

_CACHE = {}


def _mode():
    return os.environ.get("KERNEL_DT", "bf16")


def _wdt():
    return bf16 if _mode() == "bf16" else f32r


def _round_host(a):
    """Convert f32 array to the device matmul dtype on host."""
    a = np.ascontiguousarray(a, dtype=np.float32)
    if _mode() == "bf16":
        return a.astype(ml_dtypes.bfloat16)
    b = a.copy()
    v = b.view(np.uint32)
    v += 0x800
    v &= 0xFFFFF000
    return b


def _build_kernel1(bp, S, ENC, U, INDIM):
    """bp: batches per core. Returns compiled Bacc."""
    wdt = _wdt()
    EC = ENC // 128   # e-chunks
    UC = U // 128     # u-chunks
    KC = INDIM // 128  # in_dim chunks for LSTM
    NSB = S // SBLK
    G = 3 * U         # i,g,o per direction
    NBL = 2 * G // SBLK
    Tanh = mybir.ActivationFunctionType.Tanh
    Sigmoid = mybir.ActivationFunctionType.Sigmoid
    Exp = mybir.ActivationFunctionType.Exp
    Identity = mybir.ActivationFunctionType.Identity

    nc = bacc.Bacc(None)
    ENCT = nc.dram_tensor("ENCT", [bp, ENC, S], wdt, kind="ExternalInput")
    W1 = nc.dram_tensor("W1", [ENC, U], wdt, kind="ExternalInput")
    W23 = nc.dram_tensor("W23", [ENC, U], wdt, kind="ExternalInput")
    CCT = nc.dram_tensor("CCT", [ENC, bp], wdt, kind="ExternalInput")
    BQ = nc.dram_tensor("BQ", [128, UC], f32, kind="ExternalInput")
    V4 = nc.dram_tensor("V4", [128, UC], wdt, kind="ExternalInput")
    XET = nc.dram_tensor("XET", [128, (INDIM - ENC) // 128 * bp], f32,
                         kind="ExternalInput")
    KL = nc.dram_tensor("KL", [INDIM, 2 * G], wdt, kind="ExternalInput")
    BL = nc.dram_tensor("BL", [1, 2 * G], f32, kind="ExternalInput")
    HF = nc.dram_tensor("HF", [bp, U], f32, kind="ExternalOutput")
    CF = nc.dram_tensor("CF", [bp, U], f32, kind="ExternalOutput")
    HB = nc.dram_tensor("HB", [bp, U], f32, kind="ExternalOutput")
    CB = nc.dram_tensor("CB", [bp, U], f32, kind="ExternalOutput")

    with tile.TileContext(nc) as tc:
        with tc.tile_pool(name="wpool", bufs=1) as wp:
            w1_r = wp.tile([128, EC, U], wdt, tag="w1")
            nc.sync.dma_start(out=w1_r[:], in_=W1.rearrange("(c p) u -> p c u", p=128))
            v_r = wp.tile([128, UC], wdt, tag="v4")
            nc.sync.dma_start(out=v_r[:], in_=V4[:])
            bq_sb = wp.tile([128, UC], f32, tag="bq")
            nc.sync.dma_start(out=bq_sb[:], in_=BQ[:])
            qt_sb = wp.tile([128, UC * bp], f32, tag="qt")
            xin_f = wp.tile([128, KC * bp], f32, tag="xinf")
            nc.sync.dma_start(out=xin_f[:, EC * bp:KC * bp], in_=XET[:])
            bl_sb = wp.tile([1, 2 * G], f32, tag="bl")
            nc.sync.dma_start(out=bl_sb[:], in_=BL[:])
            bias_bc = wp.tile([bp, 2 * G], f32, tag="blbc")
            nc.gpsimd.partition_broadcast(bias_bc[:], bl_sb[:])
            ones_f = wp.tile([1, 128], f32, tag="ones1")
            nc.vector.memset(ones_f[:], 1.0)

            # ---- phase 1: qT = (W23.T @ ccT) + bq  -> [U, bp] as UC chunks
            with tc.tile_pool(name="p1", bufs=1) as p1, \
                 tc.tile_pool(name="psQ", bufs=2, space="PSUM") as psQ:
                w23_r = p1.tile([128, EC, U], wdt, tag="w23")
                nc.sync.dma_start(out=w23_r[:],
                                  in_=W23.rearrange("(c p) u -> p c u", p=128))
                cct_r = p1.tile([128, EC, bp], wdt, tag="cct")
                nc.sync.dma_start(out=cct_r[:],
                                  in_=CCT.rearrange("(c p) b -> p c b", p=128))
                for uc in range(UC):
                    pq = psQ.tile([128, bp], f32, tag="psq")
                    for kc in range(EC):
                        nc.tensor.matmul(pq[:], w23_r[:, kc, uc * 128:(uc + 1) * 128],
                                         cct_r[:, kc, :],
                                         start=(kc == 0), stop=(kc == EC - 1))
                    nc.scalar.activation(qt_sb[:, uc * bp:(uc + 1) * bp], pq[:],
                                         Identity, bias=bq_sb[:, uc:uc + 1],
                                         scale=1.0)

            # ---- phase 2: attention per batch
            nbuf = 2 if wdt == bf16 else 1
            with tc.tile_pool(name="encp", bufs=2) as encp, \
                 tc.tile_pool(name="thp", bufs=6) as thp, \
                 tc.tile_pool(name="prodp", bufs=nbuf) as prodp, \
                 tc.tile_pool(name="pbcp", bufs=nbuf) as pbcp, \
                 tc.tile_pool(name="rowp", bufs=2) as rowp, \
                 tc.tile_pool(name="smp", bufs=3) as smp, \
                 tc.tile_pool(name="psA", bufs=2, space="PSUM") as psA, \
                 tc.tile_pool(name="psS", bufs=2, space="PSUM") as psS, \
                 tc.tile_pool(name="psPB", bufs=1, space="PSUM") as psPB:
                for b in range(bp):
                    enc_r = encp.tile([128, EC, S], wdt, tag="enc")
                    nc.sync.dma_start(
                        out=enc_r[:],
                        in_=ENCT[b].rearrange("(c p) s -> p c s", p=128))
                    scores = rowp.tile([1, S], f32, tag="sc")
                    for sb_i in range(NSB):
                        sl = slice(sb_i * SBLK, (sb_i + 1) * SBLK)
                        ths = []
                        for uc in range(UC):
                            ph = psA.tile([128, SBLK], f32, tag="psh")
                            for kc in range(EC):
                                nc.tensor.matmul(
                                    ph[:], w1_r[:, kc, uc * 128:(uc + 1) * 128],
                                    enc_r[:, kc, sl],
                                    start=(kc == 0), stop=(kc == EC - 1))
                            th = thp.tile([128, SBLK], wdt, tag="th")
                            nc.scalar.activation(
                                th[:], ph[:], Tanh,
                                bias=qt_sb[:, uc * bp + b:uc * bp + b + 1], scale=1.0)
                            ths.append(th)
                        pss = psS.tile([1, SBLK], f32, tag="pss")
                        for uc in range(UC):
                            nc.tensor.matmul(pss[:], v_r[:, uc:uc + 1], ths[uc][:],
                                             start=(uc == 0), stop=(uc == UC - 1))
                        nc.scalar.copy(scores[:, sl], pss[:])
                    # softmax over S (partition 0 row)
                    mx = smp.tile([1, 1], f32, tag="mx")
                    nc.vector.tensor_reduce(out=mx[:], in_=scores[:],
                                            axis=mybir.AxisListType.X,
                                            op=mybir.AluOpType.max, negate=True)
                    prow = rowp.tile([1, S], f32, tag="pr")
                    es = smp.tile([1, 1], f32, tag="es")
                    nc.scalar.activation(prow[:], scores[:], Exp, bias=mx[:],
                                         scale=1.0, accum_out=es[:])
                    rinv = smp.tile([1, 1], f32, tag="rinv")
                    nc.vector.reciprocal(rinv[:], es[:])
                    rrow = smp.tile([1, 128], f32, tag="rrow")
                    nc.scalar.mul(rrow[:], ones_f[:], rinv[:])
                    ppb = psPB.tile([128, S], f32, tag="ppb")
                    for sb_i in range(NSB):
                        sl = slice(sb_i * SBLK, (sb_i + 1) * SBLK)
                        nc.tensor.matmul(ppb[:, sl], rrow[:], prow[:, sl],
                                         start=True, stop=True)
                    p_bc = pbcp.tile([128, S], wdt, tag="pbc")
                    nc.scalar.copy(p_bc[:], ppb[:])
                    # ctx: for each e-chunk, rowsum(enc * p) -> xin column
                    for ec in range(EC):
                        prod = prodp.tile([128, S], wdt, tag="prod")
                        nc.vector.tensor_tensor(out=prod[:], in0=enc_r[:, ec, :],
                                                in1=p_bc[:],
                                                op=mybir.AluOpType.mult)
                        nc.vector.tensor_reduce(
                            out=xin_f[:, ec * bp + b:ec * bp + b + 1], in_=prod[:],
                            axis=mybir.AxisListType.X, op=mybir.AluOpType.add)

            # ---- phase 3: LSTM (both directions; f-gate skipped, c0=0)
            xin_r = wp.tile([128, KC * bp], wdt, tag="xinr")
            nc.scalar.copy(xin_r[:], xin_f[:])
            with tc.tile_pool(name="klp", bufs=3) as klp, \
                 tc.tile_pool(name="gp", bufs=16) as gp, \
                 tc.tile_pool(name="psZ", bufs=2, space="PSUM") as psZ:
                kl_src = KL.rearrange("(c p) n -> p c n", p=128)
                gates = []
                for nb in range(NBL):
                    kt = klp.tile([128, KC, SBLK], wdt, tag="kl")
                    nc.sync.dma_start(
                        out=kt[:], in_=kl_src[:, :, nb * SBLK:(nb + 1) * SBLK])
                    pz = psZ.tile([bp, SBLK], f32, tag="psz")
                    for kc in range(KC):
                        nc.tensor.matmul(pz[:], xin_r[:, kc * bp:(kc + 1) * bp],
                                         kt[:, kc, :], start=(kc == 0),
                                         stop=(kc == KC - 1))
                    z_sb = gp.tile([bp, SBLK], f32, tag="g")
                    nc.vector.tensor_tensor(
                        out=z_sb[:], in0=pz[:],
                        in1=bias_bc[:, nb * SBLK:(nb + 1) * SBLK],
                        op=mybir.AluOpType.add)
                    gates.append(z_sb)
                # gate layout per direction: i (U), g (U), o (U); U = SBLK*UCg
                UCg = U // SBLK

                def gact(idx, func):
                    out = []
                    for j in range(UCg):
                        t = gp.tile([bp, SBLK], f32, tag="g")
                        nc.scalar.activation(t[:], gates[idx * UCg + j][:], func,
                                             scale=1.0)
                        out.append(t)
                    return out

                for d, (HH, CC) in enumerate(((HF, CF), (HB, CB))):
                    off = d * 3
                    i_t = gact(off + 0, Sigmoid)
                    g_t = gact(off + 1, Tanh)
                    o_t = gact(off + 2, Sigmoid)
                    for j in range(UCg):
                        c_t = gp.tile([bp, SBLK], f32, tag="g")
                        nc.vector.tensor_tensor(out=c_t[:], in0=i_t[j][:],
                                                in1=g_t[j][:],
                                                op=mybir.AluOpType.mult)
                        nc.sync.dma_start(
                            out=CC[:, j * SBLK:(j + 1) * SBLK], in_=c_t[:])
                        tc_t = gp.tile([bp, SBLK], f32, tag="g")
                        nc.scalar.activation(tc_t[:], c_t[:], Tanh, scale=1.0)
                        h_t = gp.tile([bp, SBLK], f32, tag="g")
                        nc.vector.tensor_tensor(out=h_t[:], in0=o_t[j][:],
                                                in1=tc_t[:],
                                                op=mybir.AluOpType.mult)
                        nc.sync.dma_start(
                            out=HH[:, j * SBLK:(j + 1) * SBLK], in_=h_t[:])

    nc.compile()
    return nc


def _build_kernel2(B, D2, vsh):
    """Logits kernel: [B, vsh] = outT.T @ fc_shard. D2 = 2*U."""
    wdt = _wdt()
    KC = D2 // 128
    nc = bacc.Bacc(None)
    OUTT = nc.dram_tensor("OUTT", [D2, B], wdt, kind="ExternalInput")
    FCW = nc.dram_tensor("FCW", [D2, vsh], wdt, kind="ExternalInput")
    LG = nc.dram_tensor("LG", [B, vsh], f32, kind="ExternalOutput")
    nnb = (vsh + SBLK - 1) // SBLK
    with tile.TileContext(nc) as tc:
        with tc.tile_pool(name="op", bufs=1) as op, \
             tc.tile_pool(name="fp", bufs=3) as fp, \
             tc.tile_pool(name="ob", bufs=3) as ob, \
             tc.tile_pool(name="psL", bufs=4, space="PSUM") as psL:
            outt_r = op.tile([128, KC, B], wdt, tag="outt")
            nc.sync.dma_start(out=outt_r[:],
                              in_=OUTT.rearrange("(c p) b -> p c b", p=128))
            fc_src = FCW.rearrange("(c p) v -> p c v", p=128)
            for nb in range(nnb):
                n0 = nb * SBLK
                w = min(vsh, n0 + SBLK) - n0
                ft = fp.tile([128, KC, SBLK], wdt, tag="fc")
                nc.sync.dma_start(out=ft[:, :, :w], in_=fc_src[:, :, n0:n0 + w])
                pl = psL.tile([B, SBLK], f32, tag="psl")
                for kc in range(KC):
                    nc.tensor.matmul(pl[:, :w], outt_r[:, kc, :], ft[:, kc, :w],
                                     start=(kc == 0), stop=(kc == KC - 1))
                o_sb = ob.tile([B, SBLK], f32, tag="o")
                nc.vector.tensor_copy(o_sb[:, :w], pl[:, :w])
                nc.sync.dma_start(out=LG[:, n0:n0 + w], in_=o_sb[:, :w])
    nc.compile()
    return nc


def prepare1(x, c_fwd, c_bwd, enc_output, emb,
             W1_w, W1_b, W2_w, W2_b, W3_w, W3_b, V_w, V_b,
             Kf, bf, Kb, bb, fc_w, fc_b):
    """Host prep + kernel-1 build. Returns (nc1, in_maps, dims)."""
    x = np.asarray(x)
    c_fwd = np.asarray(c_fwd, dtype=np.float32)
    c_bwd = np.asarray(c_bwd, dtype=np.float32)
    enc_output = np.asarray(enc_output, dtype=np.float32)
    emb = np.asarray(emb, dtype=np.float32)
    W1_w = np.asarray(W1_w, dtype=np.float32)
    fc_w = np.asarray(fc_w, dtype=np.float32)

    B, S, ENC = enc_output.shape
    U = W1_w.shape[1]
    EDIM = emb.shape[1]
    INDIM = ENC + EDIM
    VSZ = fc_w.shape[1]
    bp = B // NCORES
    vsh = VSZ // NCORES
    UC = U // 128
    EC = ENC // 128

    # ---------- host prep ----------
    encT = _round_host(enc_output).transpose(0, 2, 1)
    encT = np.ascontiguousarray(encT)
    xe = emb[x[:, 0].astype(np.int64)]                      # [B, EDIM]
    xin_tail = xe.T                                         # [EDIM, B]
    cct = np.concatenate([c_fwd, c_bwd], axis=1).T          # [2U, B] = [ENC, B]
    w23 = np.concatenate([np.asarray(W2_w, np.float32),
                          np.asarray(W3_w, np.float32)], axis=0)  # [2U, U]
    bq = (np.asarray(W1_b, np.float32) + np.asarray(W2_b, np.float32)
          + np.asarray(W3_b, np.float32))                   # [U]
    bq_2d = np.ascontiguousarray(bq.reshape(UC, 128).T)     # [128, UC]
    v4 = np.ascontiguousarray(np.asarray(V_w, np.float32).reshape(UC, 128).T)

    def igo(K):
        K = np.asarray(K, np.float32)
        return np.concatenate([K[:, 0:U], K[:, 2 * U:3 * U], K[:, 3 * U:4 * U]],
                              axis=1)
    kl = np.concatenate([igo(Kf), igo(Kb)], axis=1)         # [INDIM, 6U]

    def igo_b(bv):
        bv = np.asarray(bv, np.float32)
        return np.concatenate([bv[0:U], bv[2 * U:3 * U], bv[3 * U:4 * U]])
    blrow = np.concatenate([igo_b(bf), igo_b(bb)])[None, :]  # [1, 6U]

    w1_r = _round_host(W1_w)
    w23_r = _round_host(w23)
    cct_r = _round_host(cct)
    v4_r = _round_host(v4)
    kl_r = _round_host(kl)

    key1 = ("k1", bp, S, ENC, U, INDIM, _mode())
    if key1 not in _CACHE:
        _CACHE[key1] = _build_kernel1(bp, S, ENC, U, INDIM)
    nc1 = _CACHE[key1]

    ecdim = (INDIM - ENC) // 128  # xe chunks
    in_maps = []
    for c in range(NCORES):
        sl = slice(c * bp, (c + 1) * bp)
        xet = np.ascontiguousarray(
            xin_tail[:, sl].reshape(ecdim, 128, bp).transpose(1, 0, 2)
        ).reshape(128, ecdim * bp)
        in_maps.append({
            "ENCT": encT[sl],
            "W1": w1_r,
            "W23": w23_r,
            "CCT": np.ascontiguousarray(cct_r[:, sl]),
            "BQ": bq_2d,
            "V4": v4_r,
            "XET": xet,
            "KL": kl_r,
            "BL": blrow,
        })
    return nc1, in_maps, (B, U, vsh)


def prepare2(fc_w, hf, hb, B, U, vsh):
    """Kernel-2 build + in_maps from gathered LSTM outputs."""
    fc_w = np.asarray(fc_w, dtype=np.float32)
    out = np.concatenate([hf, hb], axis=1)                  # [B, 2U]
    outT_r = _round_host(np.ascontiguousarray(out.T))       # [2U, B]
    key2 = ("k2", B, 2 * U, vsh, _mode())
    if key2 not in _CACHE:
        _CACHE[key2] = _build_kernel2(B, 2 * U, vsh)
    nc2 = _CACHE[key2]
    in_maps2 = []
    for c in range(NCORES):
        fcs = _round_host(np.ascontiguousarray(fc_w[:, c * vsh:(c + 1) * vsh]))
        in_maps2.append({"OUTT": outT_r, "FCW": fcs})
    return nc2, in_maps2


def kernel(**inputs):
    nc1, in_maps, (B, U, vsh) = prepare1(**inputs)
    res1 = run_bass_kernel_spmd(nc1, in_maps, list(range(NCORES)))
    LAST_RESULTS["k1"] = res1

    hf = np.concatenate([res1.results[c]["HF"] for c in range(NCORES)], axis=0)
    cf = np.concatenate([res1.results[c]["CF"] for c in range(NCORES)], axis=0)
    hb = np.concatenate([res1.results[c]["HB"] for c in range(NCORES)], axis=0)
    cb = np.concatenate([res1.results[c]["CB"] for c in range(NCORES)], axis=0)

    nc2, in_maps2 = prepare2(inputs["fc_w"], hf, hb, B, U, vsh)
    res2 = run_bass_kernel_spmd(nc2, in_maps2, list(range(NCORES)))
    LAST_RESULTS["k2"] = res2

    logits = np.concatenate([res2.results[c]["LG"] for c in range(NCORES)],
                            axis=1)
    logits = logits + np.asarray(inputs["fc_b"], np.float32)[None, :]
    return (logits.astype(np.float32), hf, cf, hb, cb)
